# revision 14
# baseline (speedup 1.0000x reference)
# Multi-head attention (B=4, S=2048, D=512, H=8) on 8 Trainium2 NeuronCores.
#
# Sharding: core c handles batch c//2 and query rows [(c%2)*1024, (c%2+1)*1024)
# for all 8 heads over all 2048 keys. Output slices are disjoint -> no
# collectives needed.
#
# Structure (v3):
#   - masked keys compacted on host to SKC=1152 (9 tiles of 128); padding
#     lanes get a -50 exp bias. Dense fallback if a batch keeps > SKC.
#   - one software-pipelined phase: the attention loop is ACT(exp)-bound, so
#     every projection matmul group ("unit") rides the same PSUM slot
#     rotation as the score tiles, spread so the PE stays back-to-back busy
#     (TRN2 PE only reaches 2.4 GHz after ~3us of continuous execution).
#   - softmax normalization: DVE-only front (PSUM evac + denominator copy +
#     reciprocal_approx_fast) at the pair boundary, gpsimd
#     partition_broadcast, and the two multiplies deferred into the next
#     pair so the in-order DVE queue never blocks on cross-engine waits.
#   - output projection transposed (out[d, q]): bias is a per-partition
#     scalar; heads jj=0..1 are accumulated as pair-3 filler units into SBUF
#     partials, jj=2 overlaps the last normalization, jj=3 closes in PSUM
#     and a single scalar_tensor_tensor merges partial+bias+psum.

import sys
import os

for _p in ("/opt/trn_rl_repo", "/root/.axon_site/_ro/trn_rl_repo"):
    if os.path.isdir(_p) and _p not in sys.path:
        sys.path.append(_p)

import numpy as np

B, S, D, H = 4, 2048, 512, 8
DK = D // H          # 64
N_CORES = 8
SQ = S // 2          # 1024 query rows per core
SKC = 1152           # compacted key capacity (9 tiles of 128)
MASK_BIAS = -50.0

_compiled = {}       # skeys -> Bacc
last_results = None  # BassKernelResults of the most recent run (for test.py)


def _build(skeys):
    import concourse.bass as bass  # noqa: F401
    from concourse import bacc
    import concourse.tile as tile
    import concourse.mybir as mybir

    fp32 = mybir.dt.float32
    bf16 = mybir.dt.bfloat16
    add_op = mybir.AluOpType.add
    nkt = skeys // 128
    kchunks = []
    off = 0
    while off < skeys:
        w = min(512, skeys - off)
        kchunks.append((off, w))
        off += w
    nch = len(kchunks)

    nc = bacc.Bacc("TRN2", target_bir_lowering=False, debug=False,
                   num_devices=N_CORES)

    xq = nc.dram_tensor("xq", [D, SQ], bf16, kind="ExternalInput")
    xk = nc.dram_tensor("xk", [D, skeys], bf16, kind="ExternalInput")
    xv = nc.dram_tensor("xv", [D, skeys], bf16, kind="ExternalInput")
    wq = nc.dram_tensor("wq", [D, D], bf16, kind="ExternalInput")
    wk = nc.dram_tensor("wk", [D, D], bf16, kind="ExternalInput")
    wv = nc.dram_tensor("wv", [D, D], bf16, kind="ExternalInput")
    wo = nc.dram_tensor("wo", [D, D], bf16, kind="ExternalInput")
    # smalls: cols 0:4 bq, 4:8 bk, 8:12 boT, 12:12+nkt mask bias
    smalls = nc.dram_tensor("smalls", [128, 12 + nkt], fp32,
                            kind="ExternalInput")
    bvb = nc.dram_tensor("bvb", [128, D], bf16, kind="ExternalInput")
    out = nc.dram_tensor("out", [D, SQ], bf16, kind="ExternalOutput")

    with tile.TileContext(nc) as tc:
        with (
            tc.tile_pool(name="consts", bufs=1) as consts,
            tc.tile_pool(name="xin", bufs=2 + 2 * nch) as xin,
            tc.tile_pool(name="qk", bufs=1) as qk,
            tc.tile_pool(name="vp", bufs=1) as vp,
            tc.tile_pool(name="stp", bufs=8) as stp,
            tc.tile_pool(name="un", bufs=3) as un,
            tc.tile_pool(name="dnp", bufs=3) as dnp,
            tc.tile_pool(name="bcp", bufs=4) as bcp,
            tc.tile_pool(name="opp", bufs=4) as opp,
            tc.tile_pool(name="osb", bufs=2) as osb,
            tc.tile_pool(name="pst", bufs=2, space="PSUM") as pst,
            tc.tile_pool(name="pout", bufs=2, space="PSUM") as pout,
        ):
            # ---- constant / weight loads ----
            sm_sb = consts.tile([128, 12 + nkt], fp32, tag="sm")
            nc.gpsimd.dma_start(out=sm_sb[:], in_=smalls[:, :])
            bq_sb = sm_sb[:, 0:4]
            bk_sb = sm_sb[:, 4:8]
            bo_sb = sm_sb[:, 8:12]
            mb_sb = sm_sb[:, 12:12 + nkt]
            wq_sb = consts.tile([128, 4, D], bf16, tag="wq")
            wk_sb = consts.tile([128, 4, D], bf16, tag="wk")
            wv_sb = consts.tile([128, 4, D], bf16, tag="wv")
            wo_sb = consts.tile([128, 4, D], bf16, tag="wo")
            bvb_sb = consts.tile([128, D], bf16, tag="bvb")
            nc.scalar.dma_start(out=wq_sb[:],
                                in_=wq.rearrange("(kc p) n -> p kc n", p=128))
            nc.scalar.dma_start(out=wk_sb[:],
                                in_=wk.rearrange("(kc p) n -> p kc n", p=128))

            # ---- x chunk loads (all upfront; SBUF is plentiful), ordered
            # so the prologue-critical tensors are first in the queues ----
            def load_chunk(dram, off, w):
                ch = xin.tile([128, 4, 512], bf16, tag="xch", name="xch")
                nc.sync.dma_start(
                    out=ch[:, :, 0:w],
                    in_=dram[:, off:off + w]
                    .rearrange("(kc p) s -> p kc s", p=128))
                return ch

            xqch = [load_chunk(xq, 0, 512)]
            xqch.append(load_chunk(xq, 512, 512))
            xkch = [load_chunk(xk, *kchunks[0])]
            nc.sync.dma_start(out=wv_sb[:],
                              in_=wv.rearrange("(kc p) n -> p kc n", p=128))
            xvch = [load_chunk(xv, *kchunks[0])]
            xkch.append(load_chunk(xk, *kchunks[1]))
            xvch.append(load_chunk(xv, *kchunks[1]))
            nc.sync.dma_start(out=bvb_sb[:], in_=bvb[:, :])
            for c in range(2, nch):
                xkch.append(load_chunk(xk, *kchunks[c]))
                xvch.append(load_chunk(xv, *kchunks[c]))
            nc.sync.dma_start(out=wo_sb[:],
                              in_=wo.rearrange("(kc p) n -> p kc n", p=128))

            opart = [opp.tile([128, SQ], fp32, tag="op", name=f"opart_{dt}")
                     for dt in range(4)]
            qT = qk.tile([128, 4, SQ], bf16, tag="qT")
            kT = qk.tile([128, 4, skeys], bf16, tag="kT")
            outTn = qk.tile([128, 4, SQ], bf16, tag="outTn")
            v_sb = vp.tile([128, nkt, H, DK + 1], bf16, tag="v")
            nc.vector.memset(v_sb[:, :, :, DK:DK + 1], 1.0)
            # ---- PE warmup: dummy matmuls while input DMAs are in flight
            # (TRN2 PE needs ~3us of continuous work to reach 2.4 GHz) ----
            dmy = consts.tile([128, 528], bf16, tag="dmy")
            nc.vector.memset(dmy[:], 0.25)
            pdum = pst.tile([128, SQ], fp32, tag="st", name="pdum")
            for i in range(10):
                nc.tensor.matmul(pdum[0:16, 0:512], dmy[:, 0:16],
                                 dmy[:, 16:528],
                                 start=(i == 0), stop=(i == 9))

            # ---- projection units (each: one PSUM tile + matmuls + evac) --
            def opU(dt):
                # out-proj partial over head pairs jj=0..1 -> SBUF
                pt = pst.tile([128, SQ], fp32, tag="st", name=f"o_{dt}")
                for jj in range(2):
                    for qc in range(2):
                        nc.tensor.matmul(
                            pt[:, qc * 512:(qc + 1) * 512],
                            wo_sb[:, jj, dt * 128:(dt + 1) * 128],
                            outTn[:, jj, qc * 512:(qc + 1) * 512],
                            start=(jj == 0), stop=(jj == 1))
                nc.vector.tensor_copy(out=opart[dt][:], in_=pt[:])

            def qU(j):
                pt = pst.tile([128, SQ], fp32, tag="st", name=f"q_{j}")
                for qc in range(2):
                    for kc in range(4):
                        nc.tensor.matmul(
                            pt[:, qc * 512:(qc + 1) * 512],
                            wq_sb[:, kc, j * 128:(j + 1) * 128],
                            xqch[qc][:, kc, :],
                            start=(kc == 0), stop=(kc == 3))
                nc.vector.tensor_scalar_add(
                    out=qT[:, j, :], in0=pt[:], scalar1=bq_sb[:, j:j + 1])

            def kU(j, c):
                off, w = kchunks[c]
                pt = pst.tile([128, SQ], fp32, tag="st", name=f"k_{j}_{c}")
                for kc in range(4):
                    nc.tensor.matmul(
                        pt[:, 0:w],
                        wk_sb[:, kc, j * 128:(j + 1) * 128],
                        xkch[c][:, kc, 0:w],
                        start=(kc == 0), stop=(kc == 3))
                nc.vector.tensor_scalar_add(
                    out=kT[:, j, off:off + w], in0=pt[:, 0:w],
                    scalar1=bk_sb[:, j:j + 1])

            def vU(sk):
                c, i = divmod(sk, 4)
                pt = pst.tile([128, SQ], fp32, tag="st", name=f"v_{sk}")
                for kc in range(4):
                    nc.tensor.matmul(
                        pt[:, 0:512],
                        xvch[c][:, kc, i * 128:(i + 1) * 128],
                        wv_sb[:, kc, :],
                        start=(kc == 0), stop=(kc == 3))
                nc.vector.tensor_add(
                    out=v_sb[:, sk, :, 0:DK],
                    in0=pt[:, 0:512].rearrange("p (h m) -> p h m", h=H),
                    in1=bvb_sb.rearrange("p (h m) -> p h m", h=H))

            # ---- filler schedule: (pair, iter) -> unit emitters ----
            sched = {}

            def put(j, it, fn):
                sched.setdefault((j, it), []).append(fn)

            put(0, 0, lambda: vU(0))
            put(0, 0, lambda: vU(1))
            for sk in range(2, nkt):
                put(0, sk - 1, (lambda sk=sk: vU(sk)))
            put(0, 1, lambda: kU(0, 1))
            if nch > 2:
                put(0, 2, lambda: kU(0, 2))
            for c in range(3, nch):
                put(0, c, (lambda c=c: kU(0, c)))
            for j in range(1, 4):
                put(j - 1, nkt - 4, (lambda j=j: qU(j)))
                put(j - 1, nkt - 3, (lambda j=j: kU(j, 0)))
                # at the pair boundary the fresh scores tiles wait on the
                # previous pair's exps; an old-slot unit keeps the PE busy
                put(j - 1, "end", (lambda j=j: kU(j, 1)))
                if nch > 2:
                    put(j, 0, (lambda j=j: kU(j, 2)))
                for c in range(3, nch):
                    put(j, c - 1, (lambda j=j, c=c: kU(j, c)))
            for dt in range(4):
                put(3, 2 + dt, (lambda dt=dt: opU(dt)))

            # ---- prologue: just enough to start pair 0 ----
            qU(0)
            kU(0, 0)

            # ---- attention: ACT-bound inner loop with PE fillers ----
            pending_mults = None
            for j in range(4):
                poA = pout.tile([128, SQ], fp32, tag="po", name=f"poA_{j}")
                poB = pout.tile([128, SQ], fp32, tag="po", name=f"poB_{j}")
                stAs = {}
                stBs = {}
                for sk in range(nkt):
                    psA = pst.tile([128, SQ], fp32, tag="st",
                                   name=f"sA_{j}_{sk}")
                    psB = pst.tile([128, SQ], fp32, tag="st",
                                   name=f"sB_{j}_{sk}")
                    for qc in range(2):
                        nc.tensor.matmul(
                            psA[:, qc * 512:(qc + 1) * 512],
                            kT[0:DK, j, sk * 128:(sk + 1) * 128],
                            qT[0:DK, j, qc * 512:(qc + 1) * 512],
                            start=True, stop=True, tile_position=(0, 0))
                    for qc in range(2):
                        nc.tensor.matmul(
                            psB[:, qc * 512:(qc + 1) * 512],
                            kT[DK:128, j, sk * 128:(sk + 1) * 128],
                            qT[DK:128, j, qc * 512:(qc + 1) * 512],
                            start=True, stop=True, tile_position=(64, 0))
                    stA = stp.tile([128, SQ], bf16, tag="stb",
                                   name=f"eA_{j}_{sk}")
                    nc.scalar.activation(
                        out=stA[:], in_=psA[:],
                        func=mybir.ActivationFunctionType.Exp,
                        bias=mb_sb[:, sk:sk + 1], scale=0.125)
                    stB = stp.tile([128, SQ], bf16, tag="stb",
                                   name=f"eB_{j}_{sk}")
                    nc.scalar.activation(
                        out=stB[:], in_=psB[:],
                        func=mybir.ActivationFunctionType.Exp,
                        bias=mb_sb[:, sk:sk + 1], scale=0.125)
                    stAs[sk] = stA
                    stBs[sk] = stB
                    for fn in sched.get((j, sk), ()):
                        fn()
                    if sk == 5 and pending_mults is not None:
                        pending_mults()
                        pending_mults = None
                    if sk > 1:
                        for qc in range(2):
                            nc.tensor.matmul(
                                poA[0:DK + 1, qc * 512:(qc + 1) * 512],
                                v_sb[:, sk - 2, 2 * j, :],
                                stAs[sk - 2][:, qc * 512:(qc + 1) * 512],
                                start=(sk == 2), stop=False)
                        for qc in range(2):
                            nc.tensor.matmul(
                                poB[0:DK + 1, qc * 512:(qc + 1) * 512],
                                v_sb[:, sk - 2, 2 * j + 1, :],
                                stBs[sk - 2][:, qc * 512:(qc + 1) * 512],
                                start=(sk == 2), stop=False)
                for sk in (nkt - 2, nkt - 1):
                    for qc in range(2):
                        nc.tensor.matmul(
                            poA[0:DK + 1, qc * 512:(qc + 1) * 512],
                            v_sb[:, sk, 2 * j, :],
                            stAs[sk][:, qc * 512:(qc + 1) * 512],
                            start=False, stop=(sk == nkt - 1))
                    for qc in range(2):
                        nc.tensor.matmul(
                            poB[0:DK + 1, qc * 512:(qc + 1) * 512],
                            v_sb[:, sk, 2 * j + 1, :],
                            stBs[sk][:, qc * 512:(qc + 1) * 512],
                            start=False, stop=(sk == nkt - 1))
                for fn in sched.get((j, "end"), ()):
                    fn()

                if j < 3:
                    # ---- normalization front: DVE-only, no cross-engine
                    # waits inside the queue; multiplies deferred ----
                    uA = un.tile([DK + 1, SQ], fp32, tag="u", name=f"uA_{j}")
                    nc.vector.tensor_copy(out=uA[:], in_=poA[0:DK + 1, :])
                    uB = un.tile([DK + 1, SQ], fp32, tag="u", name=f"uB_{j}")
                    nc.vector.tensor_copy(out=uB[:], in_=poB[0:DK + 1, :])
                    dnA = dnp.tile([1, SQ], fp32, tag="dn", name=f"dnA_{j}")
                    dnB = dnp.tile([1, SQ], fp32, tag="dn", name=f"dnB_{j}")
                    nc.vector.tensor_copy(out=dnA[:], in_=uA[DK:DK + 1, :])
                    nc.vector.tensor_copy(out=dnB[:], in_=uB[DK:DK + 1, :])
                    rA = dnp.tile([1, SQ], fp32, tag="r", name=f"rA_{j}")
                    rB = dnp.tile([1, SQ], fp32, tag="r", name=f"rB_{j}")
                    nc.vector.reciprocal_approx_fast(out=rA[:], in_=dnA[:])
                    nc.vector.reciprocal_approx_fast(out=rB[:], in_=dnB[:])
                    bcA = bcp.tile([DK, SQ], fp32, tag="bc", name=f"bcA_{j}")
                    bcB = bcp.tile([DK, SQ], fp32, tag="bc", name=f"bcB_{j}")
                    nc.gpsimd.partition_broadcast(bcA[:], rA[:])
                    nc.gpsimd.partition_broadcast(bcB[:], rB[:])

                    def mults(j=j, uA=uA, uB=uB, bcA=bcA, bcB=bcB):
                        nc.vector.tensor_mul(out=outTn[0:DK, j, :],
                                             in0=uA[0:DK, :], in1=bcA[:])
                        nc.vector.tensor_mul(out=outTn[DK:128, j, :],
                                             in0=uB[0:DK, :], in1=bcB[:])
                    pending_mults = mults

            # ---- tail: pair-3 normalization split across DVE + ACT,
            # overlapped with out-proj jj=2 ----
            dnA = dnp.tile([1, SQ], fp32, tag="dn", name="dnA_3")
            dnB = dnp.tile([1, SQ], fp32, tag="dn", name="dnB_3")
            nc.vector.tensor_copy(out=dnA[:], in_=poA[DK:DK + 1, :])
            nc.vector.tensor_copy(out=dnB[:], in_=poB[DK:DK + 1, :])
            uA = un.tile([DK + 1, SQ], fp32, tag="u", name="uA_3")
            uB = un.tile([DK + 1, SQ], fp32, tag="u", name="uB_3")
            nc.scalar.add(out=uA[:], in_=poA[0:DK + 1, :], add=0.0)
            nc.scalar.add(out=uB[:], in_=poB[0:DK + 1, :], add=0.0)
            rA = dnp.tile([1, SQ], fp32, tag="r", name="rA_3")
            rB = dnp.tile([1, SQ], fp32, tag="r", name="rB_3")
            nc.vector.reciprocal_approx_fast(out=rA[:], in_=dnA[:])
            nc.vector.reciprocal_approx_fast(out=rB[:], in_=dnB[:])
            bcA = bcp.tile([DK, SQ], fp32, tag="bc", name="bcA_3")
            bcB = bcp.tile([DK, SQ], fp32, tag="bc", name="bcB_3")
            nc.gpsimd.partition_broadcast(bcA[:], rA[:])
            nc.gpsimd.partition_broadcast(bcB[:], rB[:])

            # out-proj jj=2 contractions fill the normalization wait
            pf = []
            for dt in range(4):
                pool, tg = (pst, "st") if dt < 2 else (pout, "po")
                pf.append(pool.tile([128, SQ], fp32, tag=tg,
                                    name=f"pf_{dt}"))
            for dt in range(4):
                for qc in range(2):
                    nc.tensor.matmul(
                        pf[dt][:, qc * 512:(qc + 1) * 512],
                        wo_sb[:, 2, dt * 128:(dt + 1) * 128],
                        outTn[:, 2, qc * 512:(qc + 1) * 512],
                        start=True, stop=False)

            nc.vector.tensor_mul(out=outTn[0:DK, 3, :],
                                 in0=uA[0:DK, :], in1=bcA[:])
            nc.vector.tensor_mul(out=outTn[DK:128, 3, :],
                                 in0=uB[0:DK, :], in1=bcB[:])

            for dt in range(4):
                for qc in range(2):
                    nc.tensor.matmul(
                        pf[dt][:, qc * 512:(qc + 1) * 512],
                        wo_sb[:, 3, dt * 128:(dt + 1) * 128],
                        outTn[:, 3, qc * 512:(qc + 1) * 512],
                        start=False, stop=True)
                ob = osb.tile([128, SQ], bf16, tag="ob", name=f"ob_{dt}")
                for qc in range(2):
                    nc.vector.scalar_tensor_tensor(
                        out=ob[:, qc * 512:(qc + 1) * 512],
                        in0=pf[dt][:, qc * 512:(qc + 1) * 512],
                        scalar=bo_sb[:, dt:dt + 1],
                        in1=opart[dt][:, qc * 512:(qc + 1) * 512],
                        op0=add_op, op1=add_op)
                    ring = nc.sync if (dt + qc) % 2 == 0 else nc.scalar
                    ring.dma_start(
                        out=out[dt * 128:(dt + 1) * 128,
                                qc * 512:(qc + 1) * 512],
                        in_=ob[:, qc * 512:(qc + 1) * 512])

    nc.finalize()
    return nc


def _get_nc(skeys):
    if skeys not in _compiled:
        _compiled[skeys] = _build(skeys)
    return _compiled[skeys]


def kernel(query, key, value, key_padding_mask, Wq, bq, Wk, bk, Wv, bv,
           Wo, bo):
    global last_results
    from concourse.bass_utils import run_bass_kernel_spmd
    import ml_dtypes
    bf = ml_dtypes.bfloat16

    query = np.asarray(query, dtype=np.float32)
    key = np.asarray(key, dtype=np.float32)
    value = np.asarray(value, dtype=np.float32)
    mask = np.asarray(key_padding_mask).astype(bool)
    Wq = np.asarray(Wq, dtype=np.float32)
    Wk = np.asarray(Wk, dtype=np.float32)
    Wv = np.asarray(Wv, dtype=np.float32)
    Wo = np.asarray(Wo, dtype=np.float32)
    bqv = np.asarray(bq, dtype=np.float32)
    bkv = np.asarray(bk, dtype=np.float32)
    bvv = np.asarray(bv, dtype=np.float32)
    bov = np.asarray(bo, dtype=np.float32)

    # compact keys: keep only unmasked positions (padded to SKC); dense
    # fallback when a batch keeps more than SKC
    kept = [np.flatnonzero(~mask[b]) for b in range(B)]
    if max(len(k) for k in kept) <= SKC:
        skeys = SKC
        kidx = []
        mbias = []
        for b in range(B):
            idx = np.zeros(SKC, dtype=np.int64)
            idx[:len(kept[b])] = kept[b]
            kidx.append(idx)
            mbias.append(np.where(np.arange(SKC) < len(kept[b]),
                                  np.float32(0.0), np.float32(MASK_BIAS)))
    else:
        skeys = S
        kidx = [None] * B
        mbias = [np.where(mask[b], np.float32(MASK_BIAS), np.float32(0.0))
                 for b in range(B)]

    nc = _get_nc(skeys)
    nkt = skeys // 128

    sm_base = np.empty((128, 12 + nkt), dtype=np.float32)
    sm_base[:, 0:4] = bqv.reshape(4, 128).T
    sm_base[:, 4:8] = bkv.reshape(4, 128).T
    sm_base[:, 8:12] = bov.reshape(4, 128).T

    shared = {
        "wq": np.ascontiguousarray(Wq.T).astype(bf),
        "wk": np.ascontiguousarray(Wk.T).astype(bf),
        "wv": np.ascontiguousarray(Wv.T).astype(bf),
        "wo": np.ascontiguousarray(Wo.T).astype(bf),
        "bvb": np.ascontiguousarray(
            np.broadcast_to(bvv[None, :], (128, D))).astype(bf),
    }
    in_maps = []
    for c in range(N_CORES):
        b, qh = divmod(c, 2)
        kc_ = key[b] if kidx[b] is None else key[b][kidx[b]]
        vc_ = value[b] if kidx[b] is None else value[b][kidx[b]]
        qT = np.ascontiguousarray(query[b].T)
        sm = sm_base.copy()
        sm[:, 12:] = mbias[b].reshape(nkt, 128).T
        m = {
            "xq": np.ascontiguousarray(
                qT[:, qh * SQ:(qh + 1) * SQ]).astype(bf),
            "xk": np.ascontiguousarray(kc_.T).astype(bf),
            "xv": np.ascontiguousarray(vc_.T).astype(bf),
            "smalls": sm,
        }
        m.update(shared)
        in_maps.append(m)

    res = run_bass_kernel_spmd(nc, in_maps, list(range(N_CORES)))
    last_results = res

    out = np.empty((B, S, D), dtype=np.float32)
    for c in range(N_CORES):
        b, qh = divmod(c, 2)
        out[b, qh * SQ:(qh + 1) * SQ, :] = \
            res.results[c]["out"].astype(np.float32).T
    return out
